# revision 13
# baseline (speedup 1.0000x reference)
"""Causal self-attention (L=4096, D=1024, 16 heads) on 8 TRN2 NeuronCores.

Sharding: tensor-parallel over heads — each core owns 2 heads (128 head-dims).
Per core:
  QT/KT = W @ x.T (+bias)          [128, L]   (head-dims on partitions)
  V     = x @ Wv.T (+bias)         [L, 128]   (tokens on partitions)
  S.T   = K @ Q.T  (per head)      [k, q] blocks, causal-skipped
  E     = exp(S.T/8) * mask        (no max-subtraction: |logits| < ~3.1)
  O.T   = [V|1].T @ E              -> unnormalized head outputs + col-sums
  O.T  /= sums  (PE broadcast + DVE reciprocal)
  partial = O @ Wo_slice.T         [L, D]
Host: out = sum_c(partial_c) + b_out.

All matmuls bf16 with fp32 PSUM accumulation (validated: scale-rel err ~4e-3).
Emission interleaves projection work for chunk g+1 into the attention i-loop
of chunk g so the PE always has fill work while ACT (the exp bottleneck)
drains, and the normalize/out-projection epilogue of chunk g-1 is deferred
into chunk g's loop head.
"""

import numpy as np
import ml_dtypes

import concourse.bass as bass
import concourse.mybir as mybir
import concourse.tile as tile
from concourse import bacc
from concourse.bass import ts
from concourse.bass_utils import run_bass_kernel_spmd

L, D = 4096, 1024
P = 128
NCORES = 8
HDC = 128          # head-dims per core (2 heads x 64)
KO = D // P        # 8 contraction chunks of the model dim
NJ = L // 512      # 8 q-chunks of 512
NK = L // P        # 32 k-chunks of 128
BF16 = mybir.dt.bfloat16
F32 = mybir.dt.float32
EXP = mybir.ActivationFunctionType.Exp


def _build():
    nc = bacc.Bacc("TRN2", target_bir_lowering=False)

    xt_d = nc.dram_tensor("xt", [D, L], BF16, kind="ExternalInput")
    wq_d = nc.dram_tensor("wq", [D, HDC], BF16, kind="ExternalInput")
    wk_d = nc.dram_tensor("wk", [D, HDC], BF16, kind="ExternalInput")
    wv_d = nc.dram_tensor("wv", [D, HDC], BF16, kind="ExternalInput")
    wo_d = nc.dram_tensor("wo", [HDC, D], BF16, kind="ExternalInput")
    bq_d = nc.dram_tensor("bq", [HDC, 1], F32, kind="ExternalInput")
    bk_d = nc.dram_tensor("bk", [HDC, 1], F32, kind="ExternalInput")
    bv_d = nc.dram_tensor("bv", [1, HDC], BF16, kind="ExternalInput")
    out_d = nc.dram_tensor("out", [L, D], BF16, kind="ExternalOutput")

    # Causal masks for the 4 diagonal-block offsets, duplicated for both heads:
    # mask[p, m, c] = 1 if (c % 512) >= p + 128*m
    qi = np.arange(512)
    half = (
        qi[None, None, :] >= (np.arange(P)[:, None, None] + 128 * np.arange(4)[None, :, None])
    )
    mask_np = np.concatenate([half, half], axis=2).astype(ml_dtypes.bfloat16)
    mask_d = nc.inline_tensor(np.ascontiguousarray(mask_np), name="maskc")
    ones1_d = nc.inline_tensor(np.ones((1, P), ml_dtypes.bfloat16), name="ones1c")
    ones64_d = nc.inline_tensor(np.ones((1, 64), ml_dtypes.bfloat16), name="ones64c")

    with tile.TileContext(nc) as tc:
        with (
            tc.tile_pool(name="const", bufs=1) as cp,
            tc.tile_pool(name="work", bufs=3) as wp,
            tc.tile_pool(name="psum", bufs=1, space="PSUM") as pp,
        ):
            # ---- small consts first so compute can start early ----
            bq = cp.tile([P, 1], F32, name="bq_s", tag="bq_s")
            bk = cp.tile([P, 1], F32, name="bk_s", tag="bk_s")
            bv = cp.tile([1, P], BF16, name="bv_s", tag="bv_s")
            nc.sync.dma_start(bq[:], bq_d[:])
            nc.sync.dma_start(bk[:], bk_d[:])
            nc.sync.dma_start(bv[:], bv_d[:])
            ones1 = cp.tile([1, P], BF16, name="ones1_s", tag="ones1_s")
            nc.sync.dma_start(ones1[:], ones1_d[:])
            ones64 = cp.tile([1, 64], BF16, name="ones64_s", tag="ones64_s")
            nc.sync.dma_start(ones64[:], ones64_d[:])
            wq = cp.tile([P, KO, HDC], BF16, name="wq_s", tag="wq_s")
            wk = cp.tile([P, KO, HDC], BF16, name="wk_s", tag="wk_s")
            wv = cp.tile([P, KO, HDC], BF16, name="wv_s", tag="wv_s")
            nc.sync.dma_start(wq[:], wq_d[:].rearrange("(ko p) m -> p ko m", p=P))
            nc.sync.dma_start(wk[:], wk_d[:].rearrange("(ko p) m -> p ko m", p=P))
            nc.sync.dma_start(wv[:], wv_d[:].rearrange("(ko p) m -> p ko m", p=P))

            # xt arrives column-group by column-group so the chunk-g projections
            # (which contract over all of D but only read token columns of g)
            # can start after ~1MB instead of the full 8MB. The mask and wo
            # loads are deferred behind the first groups (not needed earlier).
            xt = [cp.tile([P, L], BF16, name=f"xt{k}", tag=f"xt{k}") for k in range(KO)]
            maskt = cp.tile([P, 4, 1024], BF16, name="mask_s", tag="mask_s")
            wo = cp.tile([P, D], BF16, name="wo_s", tag="wo_s")
            for jcol in range(NJ):
                for k in range(KO):
                    nc.sync.dma_start(
                        xt[k][:, ts(jcol, 512)], xt_d[ts(k, P), ts(jcol, 512)]
                    )
                if jcol == 0:
                    nc.sync.dma_start(maskt[:], mask_d[:])
                if jcol == 1:
                    nc.sync.dma_start(wo[:], wo_d[:])

            qt = [cp.tile([P, 512], BF16, name=f"qt{j}", tag=f"qt{j}") for j in range(NJ)]
            kt = [cp.tile([P, 512], BF16, name=f"kt{j}", tag=f"kt{j}") for j in range(NJ)]
            ot = [cp.tile([P, 512], BF16, name=f"ot{j}", tag=f"ot{j}") for j in range(NJ)]
            v0 = [cp.tile([P, 65], BF16, name=f"v0_{i}", tag=f"v0_{i}") for i in range(NK)]
            v1 = [cp.tile([P, 65], BF16, name=f"v1_{i}", tag=f"v1_{i}") for i in range(NK)]
            for i in range(NK):
                nc.gpsimd.memset(v0[i][:, 64:65], 1.0)
                nc.gpsimd.memset(v1[i][:, 64:65], 1.0)

            # bv broadcast to all 128 partitions, done once (saves a K=1
            # matmul in every V-projection group)
            bvr = cp.tile([P, P], BF16, name="bvr_s", tag="bvr_s")
            pbv = pp.tile([P, P], F32, name="pbv", tag="mx", bufs=2)
            nc.tensor.matmul(pbv[:], ones1[:], bv[:], start=True, stop=True)
            nc.vector.tensor_copy(bvr[:], pbv[:])

            ppv = {}  # j -> (ppv0, ppv1) accumulation psums kept until epilogue

            def proj_qk(g, which):
                """QT or KT projection for token chunk g."""
                w, b, dst, nm = (wq, bq, qt, "q") if which == "q" else (wk, bk, kt, "k")
                ps = pp.tile([P, 512], F32, name=f"ps{nm}{g}", tag="mx", bufs=2)
                for k in range(KO):
                    nc.tensor.matmul(
                        ps[:], w[:, k, :], xt[k][:, ts(g, 512)],
                        start=(k == 0), stop=(k == KO - 1),
                    )
                nc.vector.tensor_scalar_add(dst[g][:], ps[:], b[:])

            def proj_v(t):
                """V projection for token tile t (both heads + bias + ones col)."""
                psv = pp.tile([P, P], F32, name=f"psv{t}", tag="mx", bufs=2)
                for k in range(KO):
                    nc.tensor.matmul(
                        psv[:], xt[k][:, ts(t, P)], wv[:, k, :],
                        start=(k == 0), stop=(k == KO - 1),
                    )
                nc.vector.tensor_tensor(
                    v0[t][:, 0:64], psv[:, 0:64], bvr[:, 0:64], mybir.AluOpType.add
                )
                nc.vector.tensor_tensor(
                    v1[t][:, 0:64], psv[:, 64:128], bvr[:, 64:128], mybir.AluOpType.add
                )

            def normalize(j):
                """Normalize chunk j's head outputs into ot[j]."""
                ppv0, ppv1 = ppv.pop(j)
                s0 = wp.tile([1, 512], BF16, name=f"s0_{j}", tag="s0", bufs=2)
                s1 = wp.tile([1, 512], BF16, name=f"s1_{j}", tag="s1", bufs=2)
                nc.vector.tensor_copy(s0[:], ppv0[64:65, :])
                nc.vector.tensor_copy(s1[:], ppv1[64:65, :])
                pb = pp.tile([P, 512], F32, name=f"pb_{j}", tag="mx", bufs=2)
                nc.tensor.matmul(pb[0:64, :], ones64[:], s0[:], start=True, stop=True)
                nc.tensor.matmul(pb[64:128, :], ones64[:], s1[:], start=True, stop=True)
                rc = wp.tile([P, 512], F32, name=f"rc_{j}", tag="rc", bufs=2)
                nc.vector.reciprocal_approx_fast(rc[:], pb[:])
                nc.vector.tensor_mul(ot[j][0:64, :], ppv0[0:64, :], rc[0:64, :])
                nc.vector.tensor_mul(ot[j][64:128, :], ppv1[0:64, :], rc[64:128, :])

            def outproj(j, t, n):
                po = pp.tile([P, 512], F32, name=f"po_{t}_{n}", tag="mx", bufs=2)
                nc.tensor.matmul(
                    po[:], ot[j][:, ts(t - 4 * j, P)], wo[:, ts(n, 512)],
                    start=True, stop=True,
                )
                ob = wp.tile([P, 512], BF16, name=f"ob_{t}_{n}", tag="ob", bufs=3)
                nc.vector.tensor_copy(ob[:], po[:])
                nc.sync.dma_start(out_d[ts(t, P), ts(n, 512)], ob[:])

            # projections for chunk 0 up front
            proj_qk(0, "q")
            proj_qk(0, "k")
            for t in range(4):
                proj_v(t)

            for g in range(NJ):
                j = g
                nkj = 4 * (j + 1)
                ppv0 = pp.tile([65, 512], F32, name=f"ppv0_{j}", tag="ppv0", bufs=1)
                ppv1 = pp.tile([65, 512], F32, name=f"ppv1_{j}", tag="ppv1", bufs=1)
                ppv[j] = (ppv0, ppv1)

                # work units spread across this i-loop: projections for chunk
                # g+1 and the out-projection of the already-normalized chunk g-1
                units = []
                if g + 1 < NJ:
                    units.append(lambda g=g: proj_qk(g + 1, "q"))
                    units.append(lambda g=g: proj_qk(g + 1, "k"))
                    for t in range(4 * g + 4, 4 * g + 8):
                        units.append(lambda t=t: proj_v(t))
                if g > 0:
                    for t in range(4 * (g - 1), 4 * (g - 1) + 4):
                        for n in range(2):
                            units.append(lambda t=t, n=n, g=g: outproj(g - 1, t, n))
                nu = len(units)
                slots = {}
                for u in range(nu):
                    slots.setdefault(min(nkj - 1, 1 + (u * nkj) // (nu + 1)), []).append(units[u])

                for i in range(nkj):
                    ps = pp.tile([P, 1024], F32, name=f"ps_{j}_{i}", tag="s", bufs=2)
                    nc.tensor.matmul(
                        ps[:, 0:512], kt[i // 4][0:64, ts(i % 4, P)], qt[j][0:64, :],
                        start=True, stop=True,
                    )
                    nc.tensor.matmul(
                        ps[:, 512:1024], kt[i // 4][64:128, ts(i % 4, P)], qt[j][64:128, :],
                        start=True, stop=True,
                    )
                    e = wp.tile([P, 1024], BF16, name=f"e_{j}_{i}", tag="e", bufs=4)
                    nc.scalar.activation(e[:], ps[:], EXP, scale=0.125)
                    m = i - 4 * j
                    if m >= 0:
                        nc.vector.tensor_mul(e[:], e[:], maskt[:, m, :])
                    if i == 0 and j > 0:
                        normalize(j - 1)
                    nc.tensor.matmul(
                        ppv0[:], v0[i][:], e[:, 0:512],
                        start=(i == 0), stop=(i == nkj - 1),
                    )
                    nc.tensor.matmul(
                        ppv1[:], v1[i][:], e[:, 512:1024],
                        start=(i == 0), stop=(i == nkj - 1),
                    )
                    for fn in slots.get(i, []):
                        fn()

            normalize(NJ - 1)
            for t in range(4 * (NJ - 1), 4 * NJ):
                for n in range(2):
                    outproj(NJ - 1, t, n)

    nc.compile()
    return nc


def _make_in_maps(x, W_qkv, b_qkv, W_out, b_out):
    bf = ml_dtypes.bfloat16
    x = np.asarray(x, np.float32)
    W_qkv = np.asarray(W_qkv, np.float32)
    b_qkv = np.asarray(b_qkv, np.float32)
    W_out = np.asarray(W_out, np.float32)
    xt = np.ascontiguousarray(x.T).astype(bf)
    in_maps = []
    for c in range(NCORES):
        r = slice(HDC * c, HDC * (c + 1))
        in_maps.append({
            "xt": xt,
            "wq": np.ascontiguousarray(W_qkv[0 * D:1 * D][r].T).astype(bf),
            "wk": np.ascontiguousarray(W_qkv[1 * D:2 * D][r].T).astype(bf),
            "wv": np.ascontiguousarray(W_qkv[2 * D:3 * D][r].T).astype(bf),
            "wo": np.ascontiguousarray(W_out[:, r].T).astype(bf),
            "bq": np.ascontiguousarray(b_qkv[0 * D:1 * D][r][:, None]).astype(np.float32),
            "bk": np.ascontiguousarray(b_qkv[1 * D:2 * D][r][:, None]).astype(np.float32),
            "bv": np.ascontiguousarray(b_qkv[2 * D:3 * D][r][None, :]).astype(bf),
        })
    return in_maps


_NC_CACHE = {}


def kernel(x, W_qkv, b_qkv, W_out, b_out):
    if "nc" not in _NC_CACHE:
        _NC_CACHE["nc"] = _build()
    nc = _NC_CACHE["nc"]
    in_maps = _make_in_maps(x, W_qkv, b_qkv, W_out, b_out)
    res = run_bass_kernel_spmd(nc, in_maps, core_ids=list(range(NCORES)))
    out = np.zeros((L, D), np.float32)
    for c in range(NCORES):
        out += res.results[c]["out"].astype(np.float32)
    out += np.asarray(b_out, np.float32)[None, :]
    return out


# revision 14
# speedup vs baseline: 1.0581x; 1.0581x over previous
"""Causal self-attention (L=4096, D=1024, 16 heads) on 8 TRN2 NeuronCores.

Sharding: tensor-parallel over heads — each core owns 2 heads (128 head-dims).
Per core:
  QT/KT = W @ x.T (+bias)          [128, L]   (head-dims on partitions)
  V     = x @ Wv.T (+bias)         [L, 128]   (tokens on partitions)
  S.T   = K @ Q.T  (per head)      [k, q] blocks, causal-skipped
  E     = exp(S.T/8) * mask        (no max-subtraction: |logits| < ~3.1)
  O.T   = [V|1].T @ E              -> unnormalized head outputs + col-sums
  O.T  /= sums  (PE broadcast + DVE reciprocal)
  partial = O @ Wo_slice.T         [L, D]
Host: out = sum_c(partial_c) + b_out.

All matmuls bf16 with fp32 PSUM accumulation (validated: scale-rel err ~4e-3).
Emission interleaves projection work for chunk g+1 into the attention i-loop
of chunk g so the PE always has fill work while ACT (the exp bottleneck)
drains, and the normalize/out-projection epilogue of chunk g-1 is deferred
into chunk g's loop head.
"""

import numpy as np
import ml_dtypes

import concourse.bass as bass
import concourse.mybir as mybir
import concourse.tile as tile
from concourse import bacc
from concourse.bass import ts
from concourse.bass_utils import run_bass_kernel_spmd

L, D = 4096, 1024
P = 128
NCORES = 8
HDC = 128          # head-dims per core (2 heads x 64)
KO = D // P        # 8 contraction chunks of the model dim
NJ = L // 512      # 8 q-chunks of 512
NK = L // P        # 32 k-chunks of 128
BF16 = mybir.dt.bfloat16
F32 = mybir.dt.float32
EXP = mybir.ActivationFunctionType.Exp


def _build():
    nc = bacc.Bacc("TRN2", target_bir_lowering=False)

    xt_d = nc.dram_tensor("xt", [D, L], BF16, kind="ExternalInput")
    wq_d = nc.dram_tensor("wq", [D, HDC], BF16, kind="ExternalInput")
    wk_d = nc.dram_tensor("wk", [D, HDC], BF16, kind="ExternalInput")
    wv_d = nc.dram_tensor("wv", [D, HDC], BF16, kind="ExternalInput")
    wo_d = nc.dram_tensor("wo", [HDC, D], BF16, kind="ExternalInput")
    bq_d = nc.dram_tensor("bq", [HDC, 1], F32, kind="ExternalInput")
    bk_d = nc.dram_tensor("bk", [HDC, 1], F32, kind="ExternalInput")
    bv_d = nc.dram_tensor("bv", [1, HDC], BF16, kind="ExternalInput")
    out_d = nc.dram_tensor("out", [L, D], BF16, kind="ExternalOutput")

    # Causal masks for the 4 diagonal-block offsets, duplicated for both heads:
    # mask[p, m, c] = 1 if (c % 512) >= p + 128*m
    qi = np.arange(512)
    half = (
        qi[None, None, :] >= (np.arange(P)[:, None, None] + 128 * np.arange(4)[None, :, None])
    )
    mask_np = np.concatenate([half, half], axis=2).astype(ml_dtypes.bfloat16)
    mask_d = nc.inline_tensor(np.ascontiguousarray(mask_np), name="maskc")
    ones1_d = nc.inline_tensor(np.ones((1, P), ml_dtypes.bfloat16), name="ones1c")
    ones64_d = nc.inline_tensor(np.ones((1, 64), ml_dtypes.bfloat16), name="ones64c")

    with tile.TileContext(nc) as tc:
        with (
            tc.tile_pool(name="const", bufs=1) as cp,
            tc.tile_pool(name="work", bufs=3) as wp,
            tc.tile_pool(name="psum", bufs=1, space="PSUM") as pp,
        ):
            # ---- small consts first so compute can start early ----
            bq = cp.tile([P, 1], F32, name="bq_s", tag="bq_s")
            bk = cp.tile([P, 1], F32, name="bk_s", tag="bk_s")
            bv = cp.tile([1, P], BF16, name="bv_s", tag="bv_s")
            nc.sync.dma_start(bq[:], bq_d[:])
            nc.sync.dma_start(bk[:], bk_d[:])
            nc.sync.dma_start(bv[:], bv_d[:])
            ones1 = cp.tile([1, P], BF16, name="ones1_s", tag="ones1_s")
            nc.sync.dma_start(ones1[:], ones1_d[:])
            ones64 = cp.tile([1, 64], BF16, name="ones64_s", tag="ones64_s")
            nc.sync.dma_start(ones64[:], ones64_d[:])
            wq = cp.tile([P, KO, HDC], BF16, name="wq_s", tag="wq_s")
            wk = cp.tile([P, KO, HDC], BF16, name="wk_s", tag="wk_s")
            wv = cp.tile([P, KO, HDC], BF16, name="wv_s", tag="wv_s")
            nc.sync.dma_start(wq[:], wq_d[:].rearrange("(ko p) m -> p ko m", p=P))
            nc.sync.dma_start(wk[:], wk_d[:].rearrange("(ko p) m -> p ko m", p=P))
            nc.sync.dma_start(wv[:], wv_d[:].rearrange("(ko p) m -> p ko m", p=P))

            # xt arrives column-group by column-group so the chunk-g projections
            # (which contract over all of D but only read token columns of g)
            # can start after ~1MB instead of the full 8MB. The mask and wo
            # loads are deferred behind the first groups (not needed earlier).
            xt = [cp.tile([P, L], BF16, name=f"xt{k}", tag=f"xt{k}") for k in range(KO)]
            maskt = cp.tile([P, 4, 1024], BF16, name="mask_s", tag="mask_s")
            wo = cp.tile([P, D], BF16, name="wo_s", tag="wo_s")
            for jcol in range(NJ):
                for k in range(KO):
                    nc.sync.dma_start(
                        xt[k][:, ts(jcol, 512)], xt_d[ts(k, P), ts(jcol, 512)]
                    )
                if jcol == 0:
                    nc.sync.dma_start(maskt[:], mask_d[:])
                if jcol == 1:
                    nc.sync.dma_start(wo[:], wo_d[:])

            qt = [cp.tile([P, 512], BF16, name=f"qt{j}", tag=f"qt{j}") for j in range(NJ)]
            kt = [cp.tile([P, 512], BF16, name=f"kt{j}", tag=f"kt{j}") for j in range(NJ)]
            ot = [cp.tile([P, 512], BF16, name=f"ot{j}", tag=f"ot{j}") for j in range(NJ)]
            v0 = [cp.tile([P, 65], BF16, name=f"v0_{i}", tag=f"v0_{i}") for i in range(NK)]
            v1 = [cp.tile([P, 65], BF16, name=f"v1_{i}", tag=f"v1_{i}") for i in range(NK)]
            for i in range(NK):
                nc.gpsimd.memset(v0[i][:, 64:65], 1.0)
                nc.gpsimd.memset(v1[i][:, 64:65], 1.0)

            # bv broadcast to all 128 partitions, done once (saves a K=1
            # matmul in every V-projection group)
            bvr = cp.tile([P, P], BF16, name="bvr_s", tag="bvr_s")
            pbv = pp.tile([P, P], F32, name="pbv", tag="mx", bufs=2)
            nc.tensor.matmul(pbv[:], ones1[:], bv[:], start=True, stop=True)
            nc.vector.tensor_copy(bvr[:], pbv[:])

            ppv = {}  # j -> (ppv0, ppv1) accumulation psums kept until epilogue

            def proj_qk(g, which):
                """QT or KT projection for token chunk g."""
                w, b, dst, nm = (wq, bq, qt, "q") if which == "q" else (wk, bk, kt, "k")
                ps = pp.tile([P, 512], F32, name=f"ps{nm}{g}", tag="mx", bufs=2)
                for k in range(KO):
                    nc.tensor.matmul(
                        ps[:], w[:, k, :], xt[k][:, ts(g, 512)],
                        start=(k == 0), stop=(k == KO - 1),
                    )
                nc.vector.tensor_scalar_add(dst[g][:], ps[:], b[:])

            def proj_v(t):
                """V projection for token tile t (both heads + bias + ones col)."""
                psv = pp.tile([P, P], F32, name=f"psv{t}", tag="mx", bufs=2)
                for k in range(KO):
                    nc.tensor.matmul(
                        psv[:], xt[k][:, ts(t, P)], wv[:, k, :],
                        start=(k == 0), stop=(k == KO - 1),
                    )
                nc.vector.tensor_tensor(
                    v0[t][:, 0:64], psv[:, 0:64], bvr[:, 0:64], mybir.AluOpType.add
                )
                nc.vector.tensor_tensor(
                    v1[t][:, 0:64], psv[:, 64:128], bvr[:, 64:128], mybir.AluOpType.add
                )

            def normalize(j):
                """Normalize chunk j's head outputs into ot[j]."""
                ppv0, ppv1 = ppv.pop(j)
                s0 = wp.tile([1, 512], BF16, name=f"s0_{j}", tag="s0", bufs=4)
                s1 = wp.tile([1, 512], BF16, name=f"s1_{j}", tag="s1", bufs=4)
                nc.vector.tensor_copy(s0[:], ppv0[64:65, :])
                nc.vector.tensor_copy(s1[:], ppv1[64:65, :])
                pb = pp.tile([P, 512], F32, name=f"pb_{j}", tag="mx", bufs=2)
                nc.tensor.matmul(pb[0:64, :], ones64[:], s0[:], start=True, stop=True)
                nc.tensor.matmul(pb[64:128, :], ones64[:], s1[:], start=True, stop=True)
                rc = wp.tile([P, 512], F32, name=f"rc_{j}", tag="rc", bufs=4)
                nc.vector.reciprocal_approx_fast(rc[:], pb[:])
                nc.vector.tensor_mul(ot[j][0:64, :], ppv0[0:64, :], rc[0:64, :])
                nc.vector.tensor_mul(ot[j][64:128, :], ppv1[0:64, :], rc[64:128, :])

            def outproj(j, t, n):
                po = pp.tile([P, 512], F32, name=f"po_{t}_{n}", tag="mx", bufs=2)
                nc.tensor.matmul(
                    po[:], ot[j][:, ts(t - 4 * j, P)], wo[:, ts(n, 512)],
                    start=True, stop=True,
                )
                ob = wp.tile([P, 512], BF16, name=f"ob_{t}_{n}", tag="ob", bufs=10)
                nc.vector.tensor_copy(ob[:], po[:])
                nc.sync.dma_start(out_d[ts(t, P), ts(n, 512)], ob[:])

            # projections for chunk 0 up front
            proj_qk(0, "q")
            proj_qk(0, "k")
            for t in range(4):
                proj_v(t)

            for g in range(NJ):
                j = g
                nkj = 4 * (j + 1)
                ppv0 = pp.tile([65, 512], F32, name=f"ppv0_{j}", tag="ppv0", bufs=1)
                ppv1 = pp.tile([65, 512], F32, name=f"ppv1_{j}", tag="ppv1", bufs=1)
                ppv[j] = (ppv0, ppv1)

                # work units spread across this i-loop: projections for chunk
                # g+1 and the out-projection of the already-normalized chunk g-1
                units = []
                if g + 1 < NJ:
                    units.append(lambda g=g: proj_qk(g + 1, "q"))
                    units.append(lambda g=g: proj_qk(g + 1, "k"))
                    for t in range(4 * g + 4, 4 * g + 8):
                        units.append(lambda t=t: proj_v(t))
                if g > 0:
                    for t in range(4 * (g - 1), 4 * (g - 1) + 4):
                        for n in range(2):
                            units.append(lambda t=t, n=n, g=g: outproj(g - 1, t, n))
                nu = len(units)
                slots = {}
                for u in range(nu):
                    slots.setdefault(min(nkj - 1, 1 + (u * nkj) // (nu + 1)), []).append(units[u])

                for i in range(nkj):
                    ps = pp.tile([P, 1024], F32, name=f"ps_{j}_{i}", tag="s", bufs=2)
                    nc.tensor.matmul(
                        ps[:, 0:512], kt[i // 4][0:64, ts(i % 4, P)], qt[j][0:64, :],
                        start=True, stop=True,
                    )
                    nc.tensor.matmul(
                        ps[:, 512:1024], kt[i // 4][64:128, ts(i % 4, P)], qt[j][64:128, :],
                        start=True, stop=True,
                    )
                    e = wp.tile([P, 1024], BF16, name=f"e_{j}_{i}", tag="e", bufs=6)
                    nc.scalar.activation(e[:], ps[:], EXP, scale=0.125)
                    m = i - 4 * j
                    if m >= 0:
                        nc.vector.tensor_mul(e[:], e[:], maskt[:, m, :])
                    if i == 0 and j > 0:
                        normalize(j - 1)
                    nc.tensor.matmul(
                        ppv0[:], v0[i][:], e[:, 0:512],
                        start=(i == 0), stop=(i == nkj - 1),
                    )
                    nc.tensor.matmul(
                        ppv1[:], v1[i][:], e[:, 512:1024],
                        start=(i == 0), stop=(i == nkj - 1),
                    )
                    for fn in slots.get(i, []):
                        fn()

            normalize(NJ - 1)
            for t in range(4 * (NJ - 1), 4 * NJ):
                for n in range(2):
                    outproj(NJ - 1, t, n)

    nc.compile()
    return nc


def _make_in_maps(x, W_qkv, b_qkv, W_out, b_out):
    bf = ml_dtypes.bfloat16
    x = np.asarray(x, np.float32)
    W_qkv = np.asarray(W_qkv, np.float32)
    b_qkv = np.asarray(b_qkv, np.float32)
    W_out = np.asarray(W_out, np.float32)
    xt = np.ascontiguousarray(x.T).astype(bf)
    in_maps = []
    for c in range(NCORES):
        r = slice(HDC * c, HDC * (c + 1))
        in_maps.append({
            "xt": xt,
            "wq": np.ascontiguousarray(W_qkv[0 * D:1 * D][r].T).astype(bf),
            "wk": np.ascontiguousarray(W_qkv[1 * D:2 * D][r].T).astype(bf),
            "wv": np.ascontiguousarray(W_qkv[2 * D:3 * D][r].T).astype(bf),
            "wo": np.ascontiguousarray(W_out[:, r].T).astype(bf),
            "bq": np.ascontiguousarray(b_qkv[0 * D:1 * D][r][:, None]).astype(np.float32),
            "bk": np.ascontiguousarray(b_qkv[1 * D:2 * D][r][:, None]).astype(np.float32),
            "bv": np.ascontiguousarray(b_qkv[2 * D:3 * D][r][None, :]).astype(bf),
        })
    return in_maps


_NC_CACHE = {}


def kernel(x, W_qkv, b_qkv, W_out, b_out):
    if "nc" not in _NC_CACHE:
        _NC_CACHE["nc"] = _build()
    nc = _NC_CACHE["nc"]
    in_maps = _make_in_maps(x, W_qkv, b_qkv, W_out, b_out)
    res = run_bass_kernel_spmd(nc, in_maps, core_ids=list(range(NCORES)))
    out = np.zeros((L, D), np.float32)
    for c in range(NCORES):
        out += res.results[c]["out"].astype(np.float32)
    out += np.asarray(b_out, np.float32)[None, :]
    return out
